# revision 44
# baseline (speedup 1.0000x reference)
import sys
import numpy as np

sys.path.insert(0, "/opt/trn_rl_repo")

from contextlib import ExitStack
from concourse import bass, bacc, tile, mybir
from concourse.bass_utils import run_bass_kernel_spmd

DT = mybir.dt.float32
DBG_HH = 0
AF = mybir.ActivationFunctionType
ALU = mybir.AluOpType
AX = mybir.AxisListType

T, D = 1024, 2048
NB, BS = 8, 128          # 128-col blocks of T
HPC = 4                  # heads per core
CPC = 256                # channels per core (HPC * 64)
NCORES = 8


def build_nc(debug=False):
    nc = bacc.Bacc(None, target_bir_lowering=False)
    h_e = nc.dram_tensor("h", [T, D], DT, kind="ExternalInput")
    wq_e = nc.dram_tensor("wq", [D, CPC], DT, kind="ExternalInput")
    wk_e = nc.dram_tensor("wk", [D, CPC], DT, kind="ExternalInput")
    wv_e = nc.dram_tensor("wv", [D, CPC], DT, kind="ExternalInput")
    ww1_e = nc.dram_tensor("ww1", [D, 32], DT, kind="ExternalInput")
    ww2_e = nc.dram_tensor("ww2", [32, CPC], DT, kind="ExternalInput")
    cw_e = nc.dram_tensor("cw", [CPC, 3], DT, kind="ExternalInput")
    wbt_e = nc.dram_tensor("wbt", [D, HPC], DT, kind="ExternalInput")
    bbt_e = nc.dram_tensor("bbt", [1, HPC], DT, kind="ExternalInput")
    wg_e = nc.dram_tensor("wg", [D, HPC], DT, kind="ExternalInput")
    bg_e = nc.dram_tensor("bg", [1, HPC], DT, kind="ExternalInput")
    wo_e = nc.dram_tensor("wo", [CPC, D], DT, kind="ExternalInput")
    ceye_e = nc.dram_tensor("ceye", [BS, BS], DT, kind="ExternalInput")
    c2eye_e = nc.dram_tensor("c2eye", [BS, BS], DT, kind="ExternalInput")
    csl_e = nc.dram_tensor("csl", [BS, BS], DT, kind="ExternalInput")
    csu_e = nc.dram_tensor("csu", [BS, BS], DT, kind="ExternalInput")
    cuti_e = nc.dram_tensor("cuti", [BS, BS], DT, kind="ExternalInput")
    cutneg_e = nc.dram_tensor("cutneg", [BS, BS], DT, kind="ExternalInput")
    chones_e = nc.dram_tensor("chones", [BS, 2], DT, kind="ExternalInput")
    chonesT_e = nc.dram_tensor("chonesT", [2, BS], DT, kind="ExternalInput")
    cones_e = nc.dram_tensor("cones", [1, BS], DT, kind="ExternalInput")
    csel_e = nc.dram_tensor("csel", [HPC, HPC * BS], DT, kind="ExternalInput")
    out_e = nc.dram_tensor("out", [T, D], DT, kind="ExternalOutput")
    if debug:
        dbg = {
            "d_qT": nc.dram_tensor("d_qT", [CPC, T], DT, kind="ExternalOutput"),
            "d_kT": nc.dram_tensor("d_kT", [CPC, T], DT, kind="ExternalOutput"),
            "d_wT": nc.dram_tensor("d_wT", [CPC, T], DT, kind="ExternalOutput"),
            "d_v": nc.dram_tensor("d_v", [T, CPC], DT, kind="ExternalOutput"),
            "d_bneg": nc.dram_tensor("d_bneg", [T, HPC], DT,
                                     kind="ExternalOutput"),
            "d_gneg": nc.dram_tensor("d_gneg", [HPC, T], DT,
                                     kind="ExternalOutput"),
            "d_P": nc.dram_tensor("d_P", [T, T], DT, kind="ExternalOutput"),
            "d_FT": nc.dram_tensor("d_FT", [T, BS], DT, kind="ExternalOutput"),
            "d_Lbd": nc.dram_tensor("d_Lbd", [T, BS], DT, kind="ExternalOutput"),
            "d_Lbd2": nc.dram_tensor("d_Lbd2", [T, BS], DT,
                                     kind="ExternalOutput"),
            "d_C": nc.dram_tensor("d_C", [T, T], DT, kind="ExternalOutput"),
            "d_A": nc.dram_tensor("d_A", [T, T], DT, kind="ExternalOutput"),
            "d_oT": nc.dram_tensor("d_oT", [CPC, T], DT,
                                   kind="ExternalOutput"),
        }

    with tile.TileContext(nc) as tc, ExitStack() as glob:
        cp = glob.enter_context(tc.tile_pool(name="consts", bufs=1))
        ceye = cp.tile([BS, BS], DT, name="ceye")
        c2eye = cp.tile([BS, BS], DT, name="c2eye")
        csl = cp.tile([BS, BS], DT, name="csl")
        csu = cp.tile([BS, BS], DT, name="csu")
        cuti = cp.tile([BS, BS], DT, name="cuti")
        cutneg = cp.tile([BS, BS], DT, name="cutneg")
        chones = cp.tile([BS, 2], DT, name="chones")
        chonesT = cp.tile([2, BS], DT, name="chonesT")
        cones = cp.tile([1, BS], DT, name="cones")
        csel = cp.tile([HPC, HPC * BS], DT, name="csel")
        for t_, e_ in ((ceye, ceye_e), (c2eye, c2eye_e), (csl, csl_e),
                       (csu, csu_e), (cuti, cuti_e), (cutneg, cutneg_e),
                       (chones, chones_e), (chonesT, chonesT_e),
                       (cones, cones_e), (csel, csel_e)):
            nc.gpsimd.dma_start(t_[:], e_[:])

        pers = glob.enter_context(tc.tile_pool(name="pers", bufs=1))
        qTs = [pers.tile([BS, T], DT, name=f"qTs{m}") for m in range(2)]
        kTs = [pers.tile([BS, T], DT, name=f"kTs{m}") for m in range(2)]
        wTs = [pers.tile([BS, T], DT, name=f"wTs{m}") for m in range(2)]
        v_sb = [pers.tile([BS, CPC], DT, name=f"vsb{m}") for m in range(NB)]
        bneg_col = [pers.tile([BS, HPC], DT, name=f"bneg{m}") for m in range(NB)]
        gneg_row = pers.tile([HPC, T], DT, name="gneg_row")
        oT_sb = [pers.tile([BS, T], DT, name=f"oTsb{m}") for m in range(2)]

        # ---------------- Phase A: load + transpose h, projections ----------------
        with ExitStack() as pa:
            wp = pa.enter_context(tc.tile_pool(name="wts", bufs=1))
            wq_sb = [wp.tile([BS, CPC], DT, name=f"wq{k}") for k in range(16)]
            wk_sb = [wp.tile([BS, CPC], DT, name=f"wk{k}") for k in range(16)]
            wv_sb = [wp.tile([BS, CPC], DT, name=f"wv{k}") for k in range(16)]
            ww1_sb = [wp.tile([BS, 32], DT, name=f"ww1{k}") for k in range(16)]
            wbt_sb = [wp.tile([BS, HPC], DT, name=f"wbt{k}") for k in range(16)]
            wg_sb = [wp.tile([BS, HPC], DT, name=f"wg{k}") for k in range(16)]
            ww2_sb = wp.tile([32, CPC], DT, name="ww2_sb")
            bbt_sb = wp.tile([1, HPC], DT, name="bbt_sb")
            bg_sb = wp.tile([1, HPC], DT, name="bg_sb")
            hT = [wp.tile([BS, T], DT, name=f"hT{k}") for k in range(16)]
            for k in range(16):
                sk = slice(k * BS, (k + 1) * BS)
                nc.gpsimd.dma_start(wq_sb[k][:], wq_e[sk, :])
                nc.gpsimd.dma_start(wk_sb[k][:], wk_e[sk, :])
                nc.gpsimd.dma_start(wv_sb[k][:], wv_e[sk, :])
                nc.gpsimd.dma_start(ww1_sb[k][:], ww1_e[sk, :])
                nc.gpsimd.dma_start(wbt_sb[k][:], wbt_e[sk, :])
                nc.gpsimd.dma_start(wg_sb[k][:], wg_e[sk, :])
            nc.gpsimd.dma_start(ww2_sb[:], ww2_e[:])
            nc.gpsimd.dma_start(bbt_sb[:], bbt_e[:])
            nc.gpsimd.dma_start(bg_sb[:], bg_e[:])

            # h natural -> hT via PE transposes
            with tc.tile_pool(name="hnat", bufs=2) as hp, \
                 tc.tile_pool(name="pst", bufs=4, space="PSUM") as pst:
                for m in range(NB):
                    h_nat = hp.tile([BS, D], DT, name="h_nat")
                    nc.gpsimd.dma_start(h_nat[:], h_e[m * BS:(m + 1) * BS, :])
                    for k in range(16):
                        ps = pst.tile([BS, BS], DT, name="ps_tr")
                        nc.tensor.transpose(ps[:], h_nat[:, k * BS:(k + 1) * BS], ceye[:])
                        nc.scalar.copy(hT[k][:, m * BS:(m + 1) * BS], ps[:])

            # q,k (chan-transposed layouts) ; w low-rank ; v natural
            with tc.tile_pool(name="psa", bufs=2, space="PSUM") as psa:
                for w_sb, dstT in ((wq_sb, qTs), (wk_sb, kTs)):
                    for mh in range(2):
                        for nh in range(2):
                            ps = psa.tile([BS, 512], DT, name="ps_qk")
                            for k in range(16):
                                nc.tensor.matmul(
                                    ps[:], w_sb[k][:, mh * BS:(mh + 1) * BS],
                                    hT[k][:, nh * 512:(nh + 1) * 512],
                                    start=(k == 0), stop=(k == 15))
                            nc.scalar.copy(dstT[mh][:, nh * 512:(nh + 1) * 512], ps[:])
                r1T = wp.tile([32, T], DT, name="r1T")
                for nh in range(2):
                    ps = psa.tile([32, 512], DT, name="ps_r1")
                    for k in range(16):
                        nc.tensor.matmul(ps[:], ww1_sb[k][:],
                                         hT[k][:, nh * 512:(nh + 1) * 512],
                                         start=(k == 0), stop=(k == 15))
                    nc.scalar.copy(r1T[:, nh * 512:(nh + 1) * 512], ps[:])
                for mh in range(2):
                    for nh in range(2):
                        ps = psa.tile([BS, 512], DT, name="ps_qk")
                        nc.tensor.matmul(ps[:], ww2_sb[:, mh * BS:(mh + 1) * BS],
                                         r1T[:, nh * 512:(nh + 1) * 512],
                                         start=True, stop=True)
                        nc.scalar.copy(wTs[mh][:, nh * 512:(nh + 1) * 512], ps[:])
                for m in range(NB):
                    ps = psa.tile([BS, CPC], DT, name="ps_v")
                    for k in range(16):
                        nc.tensor.matmul(ps[:], hT[k][:, m * BS:(m + 1) * BS],
                                         wv_sb[k][:], start=(k == 0), stop=(k == 15))
                    nc.scalar.copy(v_sb[m][:], ps[:])

            # conv + silu + l2norm on wTs
            with tc.tile_pool(name="cvp", bufs=1) as cvp, \
                 tc.tile_pool(name="pscv", bufs=2, space="PSUM") as pscv:
                cw_sb = [cvp.tile([BS, 3], DT, name=f"cw{m}") for m in range(2)]
                for m in range(2):
                    nc.gpsimd.dma_start(cw_sb[m][:], cw_e[m * BS:(m + 1) * BS, :])
                for m in range(2):
                    wcv = cvp.tile([BS, T], DT, name="wcv")
                    tsh = cvp.tile([BS, T], DT, name="tsh")
                    nc.vector.tensor_tensor(
                        wcv[:], wTs[m][:], cw_sb[m][:, 2:3].to_broadcast([BS, T]),
                        op=ALU.mult)
                    nc.vector.tensor_tensor(
                        tsh[:, :T - 1], wTs[m][:, :T - 1],
                        cw_sb[m][:, 1:2].to_broadcast([BS, T - 1]), op=ALU.mult)
                    nc.vector.tensor_tensor(wcv[:, 1:], wcv[:, 1:], tsh[:, :T - 1],
                                            op=ALU.add)
                    nc.vector.tensor_tensor(
                        tsh[:, :T - 2], wTs[m][:, :T - 2],
                        cw_sb[m][:, 0:1].to_broadcast([BS, T - 2]), op=ALU.mult)
                    nc.vector.tensor_tensor(wcv[:, 2:], wcv[:, 2:], tsh[:, :T - 2],
                                            op=ALU.add)
                    # silu (reuse tsh slot for sigmoid)
                    sg = cvp.tile([BS, T], DT, name="tsh")
                    nc.scalar.activation(sg[:], wcv[:], AF.Sigmoid)
                    nc.vector.tensor_tensor(wcv[:], wcv[:], sg[:], op=ALU.mult)
                    # l2 norm over 64-chan head groups (partition groups)
                    sq = cvp.tile([BS, T], DT, name="tsh")
                    nc.scalar.activation(sq[:], wcv[:], AF.Square)
                    ssq = cvp.tile([2, T], DT, name="ssq")
                    for nh in range(2):
                        psq = pscv.tile([2, 512], DT, name="ps_sq")
                        nc.tensor.matmul(psq[:], chones[:],
                                         sq[:, nh * 512:(nh + 1) * 512],
                                         start=True, stop=True)
                        nc.scalar.copy(ssq[:, nh * 512:(nh + 1) * 512], psq[:])
                    nc.vector.reciprocal(ssq[:], ssq[:])
                    nc.scalar.activation(ssq[:], ssq[:], AF.Sqrt)
                    rsq_bc = cvp.tile([BS, T], DT, name="rsq_bc")
                    for nh in range(2):
                        psb_ = pscv.tile([BS, 512], DT, name="ps_rb")
                        nc.tensor.matmul(psb_[:], chonesT[:],
                                         ssq[:, nh * 512:(nh + 1) * 512],
                                         start=True, stop=True)
                        nc.scalar.copy(rsq_bc[:, nh * 512:(nh + 1) * 512],
                                       psb_[:])
                    nc.vector.tensor_tensor(wTs[m][:], wcv[:], rsq_bc[:],
                                            op=ALU.mult)

            # beta / g / G rows
            with tc.tile_pool(name="bgp", bufs=2) as bgp, \
                 tc.tile_pool(name="psb", bufs=2, space="PSUM") as psb:
                grow = pers.tile([HPC, T], DT, name="grow")
                lsg_col = [bgp.tile([BS, HPC], DT, name=f"lsg{m}", bufs=8)
                           for m in range(NB)]
                bbt_bc = bgp.tile([BS, HPC], DT, name="bbt_bc")
                bg_bc = bgp.tile([BS, HPC], DT, name="bg_bc")
                for bc_, src_ in ((bbt_bc, bbt_sb), (bg_bc, bg_sb)):
                    psbc = psb.tile([BS, HPC], DT, name="ps_bc")
                    nc.tensor.matmul(psbc[:], cones[:], src_[:],
                                     start=True, stop=True)
                    nc.scalar.copy(bc_[:], psbc[:])
                for m in range(NB):
                    sm = slice(m * BS, (m + 1) * BS)
                    psbt = psb.tile([BS, HPC], DT, name="ps_bt")
                    for k in range(16):
                        nc.tensor.matmul(psbt[:], hT[k][:, sm], wbt_sb[k][:],
                                         start=(k == 0), stop=(k == 15))
                    tb = bgp.tile([BS, HPC], DT, name="tb")
                    nc.vector.tensor_tensor(tb[:], psbt[:], bbt_bc[:],
                                            op=ALU.add)
                    sgb = bgp.tile([BS, HPC], DT, name="sgb")
                    nc.scalar.activation(sgb[:], tb[:], AF.Sigmoid)
                    nc.vector.tensor_scalar_mul(bneg_col[m][:], sgb[:], -2.0)

                    psg = psb.tile([BS, HPC], DT, name="ps_g")
                    for k in range(16):
                        nc.tensor.matmul(psg[:], hT[k][:, sm], wg_sb[k][:],
                                         start=(k == 0), stop=(k == 15))
                    tg = bgp.tile([BS, HPC], DT, name="tg")
                    nc.vector.tensor_tensor(tg[:], psg[:], bg_bc[:],
                                            op=ALU.add)
                    sgg = bgp.tile([BS, HPC], DT, name="sgg")
                    nc.scalar.activation(sgg[:], tg[:], AF.Sigmoid)
                    nc.scalar.activation(lsg_col[m][:], sgg[:], AF.Ln)
                # cumsum -> grow ; prefix adds ; negate -> gneg_row
                for m in range(NB):
                    psc = psb.tile([HPC, BS], DT, name="ps_cum")
                    nc.tensor.matmul(psc[:], lsg_col[m][:], cuti[:],
                                     start=True, stop=True)
                    nc.scalar.copy(grow[:, m * BS:(m + 1) * BS], psc[:])
                for m in range(1, NB):
                    nc.vector.tensor_tensor(
                        grow[:, m * BS:(m + 1) * BS],
                        grow[:, m * BS:(m + 1) * BS],
                        grow[:, m * BS - 1:m * BS].to_broadcast([HPC, BS]),
                        op=ALU.add)
                nc.vector.tensor_scalar_mul(gneg_row[:], grow[:], -1.0)

        if debug:
            for m in range(2):
                sm = slice(m * BS, (m + 1) * BS)
                nc.gpsimd.dma_start(dbg["d_qT"][sm, :], qTs[m][:])
                nc.gpsimd.dma_start(dbg["d_kT"][sm, :], kTs[m][:])
                nc.gpsimd.dma_start(dbg["d_wT"][sm, :], wTs[m][:])
            for m in range(NB):
                sm = slice(m * BS, (m + 1) * BS)
                nc.gpsimd.dma_start(dbg["d_v"][sm, :], v_sb[m][:])
                nc.gpsimd.dma_start(dbg["d_bneg"][sm, :], bneg_col[m][:])
            nc.gpsimd.dma_start(dbg["d_gneg"][:], gneg_row[:])

        # ---------------- Phase B: per-head attention ----------------
        for hh in range(HPC):
            mt = hh // 2
            pof = (hh % 2) * 64

            def wTh(i):
                return wTs[mt][pof:pof + 64, i * BS:(i + 1) * BS]

            def qTh(i):
                return qTs[mt][pof:pof + 64, i * BS:(i + 1) * BS]

            def kTh(i):
                return kTs[mt][pof:pof + 64, i * BS:(i + 1) * BS]

            def bnb(j):
                return bneg_col[j][:, hh:hh + 1].to_broadcast([BS, BS])

            with ExitStack() as ph:
                hb = ph.enter_context(tc.tile_pool(name=f"hb{hh}", bufs=1))
                hsc = ph.enter_context(tc.tile_pool(name=f"hsc{hh}", bufs=3))
                Lb, Rb, C = {}, {}, {}
                Sd, FT = {}, {}
                Pex = [hb.tile([BS, T], DT, name=f"Pex{i}") for i in range(NB)]
                rs = hb.tile([BS, NB], DT, name="rs")
                gneg_bc = hb.tile([BS, T], DT, name="gneg_bc")

                # pairwise blocks
                with tc.tile_pool(name="pspw", bufs=2, space="PSUM") as pspw:
                    for nh in range(2):
                        psgb = pspw.tile([BS, 512], DT, name="ps_gb")
                        nc.tensor.matmul(
                            psgb[:], csel[:, hh * BS:(hh + 1) * BS],
                            gneg_row[:, nh * 512:(nh + 1) * 512],
                            start=True, stop=True)
                        nc.scalar.copy(gneg_bc[:, nh * 512:(nh + 1) * 512],
                                       psgb[:])
                    for i in range(NB):
                        for j in range(i + 1):
                            psL = pspw.tile([BS, BS], DT, name="ps_L")
                            nc.tensor.matmul(psL[:], wTh(j), wTh(i),
                                             start=True, stop=True)
                            Lb[(j, i)] = hb.tile([BS, BS], DT, name=f"Lb{j}_{i}")
                            nc.vector.tensor_tensor(Lb[(j, i)][:], psL[:], bnb(j),
                                                    op=ALU.mult)
                            psR = pspw.tile([BS, BS], DT, name="ps_R")
                            nc.tensor.matmul(psR[:], wTh(j), qTh(i),
                                             start=True, stop=True)
                            Rb[(j, i)] = hb.tile([BS, BS], DT, name=f"Rb{j}_{i}")
                            nc.vector.tensor_tensor(Rb[(j, i)][:], psR[:], bnb(j),
                                                    op=ALU.mult)
                            if j == i:
                                nc.vector.tensor_tensor(Rb[(j, i)][:], Rb[(j, i)][:],
                                                        cuti[:], op=ALU.mult)
                                psS = pspw.tile([BS, BS], DT, name="ps_S")
                                nc.tensor.matmul(psS[:], wTh(i), kTh(i),
                                                 start=True, stop=True)
                                Sd[i] = hb.tile([BS, BS], DT, name=f"Sd{i}")
                                nc.vector.tensor_tensor(Sd[i][:], psS[:], csl[:],
                                                        op=ALU.mult)
                                if debug and hh == DBG_HH:
                                    nc.gpsimd.dma_start(
                                        dbg["d_Lbd"][i * BS:(i + 1) * BS, :],
                                        Lb[(i, i)][:])

                # Newton inversion of diagonal blocks -> FT[i]
                with tc.tile_pool(name="psnt", bufs=3, space="PSUM") as psnt:
                    for i in range(NB):
                        if debug and hh == DBG_HH:
                            nc.gpsimd.dma_start(
                                dbg["d_Lbd2"][i * BS:(i + 1) * BS, :],
                                Lb[(i, i)][:])
                        t0 = hsc.tile([BS, BS], DT, name="nt_t0")
                        nc.vector.tensor_tensor(t0[:], Lb[(i, i)][:], csl[:],
                                                op=ALU.mult)
                        F = hsc.tile([BS, BS], DT, name="nt_F")
                        nc.vector.tensor_tensor(F[:], t0[:], ceye[:], op=ALU.add)
                        t1 = hsc.tile([BS, BS], DT, name="nt_t1")
                        nc.vector.tensor_tensor(t1[:], Lb[(i, i)][:], csu[:],
                                                op=ALU.mult)
                        U = hsc.tile([BS, BS], DT, name="nt_U")
                        nc.vector.tensor_tensor(U[:], ceye[:], t1[:],
                                                op=ALU.subtract)
                        pstr = psnt.tile([BS, BS], DT, name="ps_nt")
                        nc.tensor.transpose(pstr[:], F[:], ceye[:])
                        FTc = hsc.tile([BS, BS], DT, name="nt_FT")
                        nc.scalar.copy(FTc[:], pstr[:])
                        for it in range(8):
                            last = (it == 7)
                            psG = psnt.tile([BS, BS], DT, name="ps_nt")
                            nc.tensor.matmul(psG[:], U[:], F[:],
                                             start=True, stop=True)
                            Hh = hsc.tile([BS, BS], DT, name="nt_H")
                            nc.vector.tensor_tensor(Hh[:], c2eye[:], psG[:],
                                                    op=ALU.subtract)
                            if not last:
                                psF = psnt.tile([BS, BS], DT, name="ps_nt")
                                nc.tensor.matmul(psF[:], FTc[:], Hh[:],
                                                 start=True, stop=True)
                            psFT = psnt.tile([BS, BS], DT, name="ps_nt")
                            nc.tensor.matmul(psFT[:], Hh[:], FTc[:],
                                             start=True, stop=True)
                            if not last:
                                F = hsc.tile([BS, BS], DT, name="nt_F")
                                nc.scalar.copy(F[:], psF[:])
                                FTc = hsc.tile([BS, BS], DT, name="nt_FT")
                                nc.scalar.copy(FTc[:], psFT[:])
                            else:
                                FT[i] = hb.tile([BS, BS], DT, name=f"FT{i}")
                                nc.scalar.copy(FT[i][:], psFT[:])
                        if debug and hh == DBG_HH:
                            nc.gpsimd.dma_start(
                                dbg["d_FT"][i * BS:(i + 1) * BS, :], FT[i][:])

                # blocked forward solve -> C[(i,c)]
                with tc.tile_pool(name="pssv", bufs=2, space="PSUM") as pssv:
                    for i in range(NB):
                        for c in range(i + 1):
                            if c == i:
                                ysrc = Sd[i]
                            else:
                                psY = pssv.tile([BS, BS], DT, name="ps_y")
                                nc.tensor.matmul(psY[:], wTh(i), kTh(c),
                                                 start=True, stop=False)
                                for j in range(c, i):
                                    nc.tensor.matmul(psY[:], Lb[(j, i)][:],
                                                     C[(j, c)][:],
                                                     start=False, stop=(j == i - 1))
                                ysrc = hsc.tile([BS, BS], DT, name="ysb")
                                nc.scalar.copy(ysrc[:], psY[:])
                            psC = pssv.tile([BS, BS], DT, name="ps_c")
                            nc.tensor.matmul(psC[:], FT[i][:], ysrc[:],
                                             start=True, stop=True)
                            C[(i, c)] = hb.tile([BS, BS], DT, name=f"C{i}_{c}")
                            nc.scalar.copy(C[(i, c)][:], psC[:])
                            if debug and hh == DBG_HH:
                                nc.gpsimd.dma_start(
                                    dbg["d_C"][i * BS:(i + 1) * BS,
                                               c * BS:(c + 1) * BS],
                                    C[(i, c)][:])

                # A blocks + softmax
                with tc.tile_pool(name="psat", bufs=3, space="PSUM") as psat:
                    for i in range(NB):
                        for c in range(i + 1):
                            psA = psat.tile([BS, BS], DT, name="ps_A")
                            nc.tensor.matmul(psA[:], qTh(i), kTh(c),
                                             start=True, stop=False)
                            for l in range(c, i + 1):
                                nc.tensor.matmul(psA[:], Rb[(l, i)][:],
                                                 C[(l, c)][:],
                                                 start=False, stop=(l == i))
                            nc.scalar.mul(Pex[i][:, c * BS:(c + 1) * BS], psA[:],
                                          0.125)
                            if debug and hh == DBG_HH:
                                nc.gpsimd.dma_start(
                                    dbg["d_A"][i * BS:(i + 1) * BS,
                                               c * BS:(c + 1) * BS],
                                    Pex[i][:, c * BS:(c + 1) * BS])
                        used = (i + 1) * BS
                        row = Pex[i][:, :used]
                        nc.vector.tensor_tensor(row, row, gneg_bc[:, :used],
                                                op=ALU.add)
                        nc.vector.tensor_tensor(Pex[i][:, i * BS:used],
                                                Pex[i][:, i * BS:used],
                                                cutneg[:], op=ALU.add)
                        mx = hsc.tile([BS, 1], DT, name="mx")
                        nc.vector.tensor_reduce(mx[:], row, axis=AX.X, op=ALU.max)
                        negmx = hsc.tile([BS, 1], DT, name="negmx")
                        nc.vector.tensor_scalar_mul(negmx[:], mx[:], -1.0)
                        sums = hsc.tile([BS, 1], DT, name="sums")
                        nc.scalar.activation(row, row, AF.Exp, bias=negmx[:],
                                             scale=1.0, accum_out=sums[:])
                        nc.vector.reciprocal(rs[:, i:i + 1], sums[:])
                        nc.vector.tensor_tensor(
                            row, row, rs[:, i:i + 1].to_broadcast([BS, used]),
                            op=ALU.mult)
                        if debug and hh == DBG_HH:
                            nc.gpsimd.dma_start(
                                dbg["d_P"][i * BS:(i + 1) * BS, :used], row)

                # P @ v  -> oT  (transpose P blocks, accumulate)
                with tc.tile_pool(name="pspv", bufs=2, space="PSUM") as pspv, \
                     tc.tile_pool(name="pxp", bufs=9) as pxp:
                    for i in range(NB):
                        pxs = []
                        for c in range(i + 1):
                            pstp = pspv.tile([BS, BS], DT, name="ps_pt")
                            nc.tensor.transpose(pstp[:],
                                                Pex[i][:, c * BS:(c + 1) * BS],
                                                ceye[:])
                            px = pxp.tile([BS, BS], DT, name="pexT")
                            nc.scalar.copy(px[:], pstp[:])
                            pxs.append(px)
                        pso = pspv.tile([64, BS], DT, name="ps_o")
                        for c in range(i + 1):
                            nc.tensor.matmul(pso[:],
                                             v_sb[c][:, hh * 64:(hh + 1) * 64],
                                             pxs[c][:],
                                             start=(c == 0), stop=(c == i))
                        nc.scalar.copy(
                            oT_sb[mt][pof:pof + 64, i * BS:(i + 1) * BS],
                            pso[:])

        if debug:
            for m in range(2):
                nc.gpsimd.dma_start(dbg["d_oT"][m * BS:(m + 1) * BS, :],
                                    oT_sb[m][:])

        # ---------------- Phase C: output projection ----------------
        with tc.tile_pool(name="wop", bufs=1) as wop, \
             tc.tile_pool(name="outp", bufs=2) as outp, \
             tc.tile_pool(name="psf", bufs=2, space="PSUM") as psf:
            wo_sb = [wop.tile([BS, D], DT, name=f"wo{m}") for m in range(2)]
            for m in range(2):
                nc.gpsimd.dma_start(wo_sb[m][:], wo_e[m * BS:(m + 1) * BS, :])
            for m in range(NB):
                ot = outp.tile([BS, D], DT, name="ot")
                for n in range(4):
                    ps = psf.tile([BS, 512], DT, name="ps_f")
                    for cchunk in range(2):
                        nc.tensor.matmul(ps[:],
                                         oT_sb[cchunk][:, m * BS:(m + 1) * BS],
                                         wo_sb[cchunk][:, n * 512:(n + 1) * 512],
                                         start=(cchunk == 0), stop=(cchunk == 1))
                    nc.scalar.copy(ot[:, n * 512:(n + 1) * 512], ps[:])
                nc.gpsimd.dma_start(out_e[m * BS:(m + 1) * BS, :], ot[:])

    nc.finalize()
    return nc


_NC = None


def _get_nc():
    global _NC
    if _NC is None:
        _NC = build_nc()
    return _NC


def _consts():
    eye = np.eye(BS, dtype=np.float32)
    sl = np.tril(np.ones((BS, BS), np.float32), -1)
    su = sl.T.copy()
    uti = np.triu(np.ones((BS, BS), np.float32))
    utneg = (su * np.float32(-1e30)).astype(np.float32)
    hones = np.zeros((BS, 2), np.float32)
    hones[:64, 0] = 1.0
    hones[64:, 1] = 1.0
    honesT = np.ascontiguousarray(hones.T)
    ones_row = np.ones((1, BS), np.float32)
    sel = np.zeros((HPC, HPC * BS), np.float32)
    for hh in range(HPC):
        sel[hh, hh * BS:(hh + 1) * BS] = 1.0
    return (eye, (2 * eye).astype(np.float32), sl, su, uti, utneg, hones,
            honesT, ones_row, sel)


def _in_maps(inputs):
    f32 = lambda a: np.ascontiguousarray(np.asarray(a), dtype=np.float32)
    h = f32(inputs["hidden_states"]).reshape(T, D)
    Wq, Wk, Wv = f32(inputs["Wq"]), f32(inputs["Wk"]), f32(inputs["Wv"])
    Ww1, Ww2 = f32(inputs["Ww1"]), f32(inputs["Ww2"])
    cw = f32(inputs["conv_w"])
    Wbt, bbt = f32(inputs["Wbt"]), f32(inputs["bbt"])
    Wg, bg = f32(inputs["Wg"]), f32(inputs["bg"])
    Wo = f32(inputs["Wo"])
    eye, e2, sl, su, uti, utneg, hones, honesT, ones_row, sel = _consts()
    maps = []
    for core in range(NCORES):
        cs = slice(core * CPC, (core + 1) * CPC)
        hs = slice(core * HPC, (core + 1) * HPC)
        maps.append({
            "h": h,
            "wq": np.ascontiguousarray(Wq[:, cs]),
            "wk": np.ascontiguousarray(Wk[:, cs]),
            "wv": np.ascontiguousarray(Wv[:, cs]),
            "ww1": Ww1,
            "ww2": np.ascontiguousarray(Ww2[:, cs]),
            "cw": np.ascontiguousarray(cw[cs]),
            "wbt": np.ascontiguousarray(Wbt[:, hs]),
            "bbt": np.ascontiguousarray(bbt[hs].reshape(1, HPC)),
            "wg": np.ascontiguousarray(Wg[:, hs]),
            "bg": np.ascontiguousarray(bg[hs].reshape(1, HPC)),
            "wo": np.ascontiguousarray(Wo[cs, :]),
            "ceye": eye, "c2eye": e2, "csl": sl, "csu": su,
            "cuti": uti, "cutneg": utneg, "chones": hones,
            "chonesT": honesT, "cones": ones_row, "csel": sel,
        })
    return maps


LAST_RESULT = None


def kernel(**inputs):
    global LAST_RESULT
    import os
    nc = _get_nc()
    maps = _in_maps(inputs)
    trace = bool(int(os.environ.get("KERNEL_TRACE", "0")))
    res = run_bass_kernel_spmd(nc, maps, list(range(NCORES)), trace=trace)
    LAST_RESULT = res
    acc = None
    for r in res.results:
        o = np.asarray(r["out"], dtype=np.float32)
        acc = o if acc is None else acc + o
    return acc.reshape(1, T, D)


if __name__ == "__main__":
    nc = build_nc()
    n_inst = sum(len(bb.instructions) for bb in nc.main_func.blocks)
    print("built ok, instructions:", n_inst)
